# revision 1
# baseline (speedup 1.0000x reference)
"""Trainium2 Bass kernel for the CustomGRU cell — v3.

v3 = v2 (fp16 I/O, Act-centric layout) + software-pipelined blend:
the DVE blend (d, zd, ht) of chunk i is emitted after chunk i+1's rh
multiply, so sigma_r(i+1) -> rh(i+1) -> ph(i+1) U-matmuls never queue
behind blend work and tanh(i+1) starts the moment the Act engine is free.
Last chunk blends at 512-col granularity with quarter stores to shorten
the drain tail.
"""

import numpy as np

import concourse.bacc as bacc
import concourse.mybir as mybir
import concourse.tile as tile
from concourse.bass_utils import run_bass_kernel_spmd

N_CORES = 8
B_FULL = 262144
D = 128
B_LOC = B_FULL // N_CORES  # 32768 rows per core

F32 = mybir.dt.float32
F16 = mybir.dt.float16
AF = mybir.ActivationFunctionType


def build_gru(nc, b_loc, chunk=2048, sub=512, nrep=1, io_bufs=4, mid_bufs=3,
              psg=1024, n_warm=8):
    xt = nc.dram_tensor("xt", [D, b_loc], F16, kind="ExternalInput").ap()
    ht = nc.dram_tensor("ht", [D, b_loc], F16, kind="ExternalInput").ap()
    wa = nc.dram_tensor("w_all", [5 * D, D], F16, kind="ExternalInput").ap()
    ba = nc.dram_tensor("b_all", [D, 3], F32, kind="ExternalInput").ap()
    hto = nc.dram_tensor("ht_out", [D, b_loc], F16, kind="ExternalOutput").ap()
    hco = nc.dram_tensor("hc_out", [D, b_loc], F16, kind="ExternalOutput").ap()

    n_chunks = b_loc // chunk
    n_sub = chunk // sub
    n_total = n_chunks * nrep

    with tile.TileContext(nc) as tc:
        with (
            tc.tile_pool(name="w", bufs=1) as wpool,
            tc.tile_pool(name="io", bufs=io_bufs) as io,
            tc.tile_pool(name="mid", bufs=mid_bufs) as mid,
            tc.tile_pool(name="ps", bufs=4096 // psg, space="PSUM") as ps,
        ):
            # DMA order tuned for the first sigma_r: W_r+U_r and the bias
            # first, then the first x/h halves, then the remaining weights.
            w = [None] * 5
            for k in (2,):
                t = wpool.tile([D, D], F16, tag=f"w{k}")
                nc.sync.dma_start(t[:], wa[k * D:(k + 1) * D, :])
                w[k] = t[:]
            bt = wpool.tile([D, 3], F32, tag="b")
            nc.sync.dma_start(bt[:], ba[:, :])
            first_x = io.tile([D, chunk], F16, tag="x")
            half = chunk // 2
            nc.sync.dma_start(first_x[:, 0:half], xt[:, 0:half])
            first_h = io.tile([D, chunk], F16, tag="h")
            nc.sync.dma_start(first_h[:, 0:half], ht[:, 0:half])
            nc.sync.dma_start(first_x[:, half:chunk], xt[:, half:chunk])
            nc.sync.dma_start(first_h[:, half:chunk], ht[:, half:chunk])
            for k in (0, 1, 3, 4):
                t = wpool.tile([D, D], F16, tag=f"w{k}")
                nc.sync.dma_start(t[:], wa[k * D:(k + 1) * D, :])
                w[k] = t[:]

            # Act warmup: memset a tiny tile and run sigmoid+tanh on it so the
            # activation table (sigmoid_and_others covers both) loads before
            # any data arrives.
            warm = wpool.tile([D, 8], F32, tag="warm")
            nc.vector.memset(warm[:], 0.0)
            warm_o = wpool.tile([D, 8], F16, tag="warm_o")
            nc.scalar.activation(warm_o[:], warm[:], AF.Sigmoid, bias=0.0)
            nc.scalar.activation(warm_o[:], warm[:], AF.Tanh, bias=0.0)

            # PE p-state warmup: stream weight tiles as moving operands into a
            # junk PSUM tile. ~30 x 128-col matmuls ~ 6us at low clock, enough
            # to carry the PE past the 3us full-clock ramp threshold while the
            # first x/h chunk is still in flight.
            if n_warm:
                pwarm = ps.tile([D, psg], F32, tag="p")
            for wi in range(n_warm):
                sl = slice((wi % (psg // D)) * D, (wi % (psg // D) + 1) * D)
                nc.tensor.matmul(pwarm[:, sl], w[2], w[2],
                                 start=True, stop=True)

            # pending blend state from the previous chunk:
            pend = None  # (lo, hi, hs, z_s, hcs)

            def emit_blend(state, granular):
                lo_p, hi_p, hs_p, z_p, hc_p = state
                hts = io.tile([D, chunk], F16, tag="hto")
                if not granular:
                    d_s = mid.tile([D, chunk], F16, tag="d")
                    nc.vector.tensor_sub(d_s[:], hc_p[:], hs_p[:])
                    zd_s = mid.tile([D, chunk], F16, tag="zd")
                    nc.vector.tensor_mul(zd_s[:], z_p[:], d_s[:])
                    nc.vector.tensor_add(hts[:], hs_p[:], zd_s[:])
                    nc.gpsimd.dma_start(hto[:, lo_p:hi_p], hts[:])
                else:
                    # final chunk: 512-col granularity so stores drain early;
                    # alternate SP/Pool DGE so triggers don't serialize
                    d_s = mid.tile([D, chunk], F16, tag="d")
                    zd_s = mid.tile([D, chunk], F16, tag="zd")
                    for qi in range(n_sub):
                        q = slice(qi * sub, (qi + 1) * sub)
                        g = slice(lo_p + qi * sub, lo_p + (qi + 1) * sub)
                        nc.vector.tensor_sub(d_s[:, q], hc_p[:, q], hs_p[:, q])
                        nc.vector.tensor_mul(zd_s[:, q], z_p[:, q], d_s[:, q])
                        nc.vector.tensor_add(hts[:, q], hs_p[:, q], zd_s[:, q])
                        eng = nc.sync if qi % 2 == 0 else nc.gpsimd
                        eng.dma_start(hto[:, g], hts[:, q])

            def emit_tail_chunk(lo, hi, xs, hs):
                """Last chunk: normal gate pipeline, but blend + store follow
                each tanh granule immediately; final granule at 512 cols."""
                n_pg = chunk // psg
                r_s = mid.tile([D, chunk], F16, tag="r")
                z_s = mid.tile([D, chunk], F16, tag="z")
                hcs = io.tile([D, chunk], F16, tag="hco")
                hts = io.tile([D, chunk], F16, tag="hto")
                d_s = mid.tile([D, chunk], F16, tag="d")
                zd_s = mid.tile([D, chunk], F16, tag="zd")
                for gi in range(n_pg):
                    g = slice(gi * psg, (gi + 1) * psg)
                    pr = ps.tile([D, psg], F32, tag="p")
                    for si in range(psg // sub):
                        sl = slice(gi * psg + si * sub, gi * psg + (si + 1) * sub)
                        pl = slice(si * sub, (si + 1) * sub)
                        nc.tensor.matmul(pr[:, pl], w[2], xs[:, sl],
                                         start=True, stop=True)
                    nc.scalar.activation(r_s[:, g], pr[:], AF.Sigmoid,
                                         bias=bt[:, 1:2])
                nc.vector.tensor_mul(rh_tail[:], r_s[:], hs[:])
                for gi in range(n_pg):
                    g = slice(gi * psg, (gi + 1) * psg)
                    pz = ps.tile([D, psg], F32, tag="p")
                    for si in range(psg // sub):
                        sl = slice(gi * psg + si * sub, gi * psg + (si + 1) * sub)
                        pl = slice(si * sub, (si + 1) * sub)
                        nc.tensor.matmul(pz[:, pl], w[0], xs[:, sl],
                                         start=True, stop=False)
                        nc.tensor.matmul(pz[:, pl], w[1], hs[:, sl],
                                         start=False, stop=True)
                    nc.scalar.activation(z_s[:, g], pz[:], AF.Sigmoid,
                                         bias=bt[:, 0:1])
                for gi in range(n_pg):
                    g = slice(gi * psg, (gi + 1) * psg)
                    gg = slice(lo + gi * psg, lo + (gi + 1) * psg)
                    ph = ps.tile([D, psg], F32, tag="p")
                    for si in range(psg // sub):
                        sl = slice(gi * psg + si * sub, gi * psg + (si + 1) * sub)
                        pl = slice(si * sub, (si + 1) * sub)
                        nc.tensor.matmul(ph[:, pl], w[3], xs[:, sl],
                                         start=True, stop=False)
                        nc.tensor.matmul(ph[:, pl], w[4], rh_tail[:, sl],
                                         start=False, stop=True)
                    nc.scalar.activation(hcs[:, g], ph[:], AF.Tanh,
                                         bias=bt[:, 2:3])
                    nc.sync.dma_start(hco[:, gg], hcs[:, g])
                    nc.vector.tensor_sub(d_s[:, g], hcs[:, g], hs[:, g])
                    nc.vector.tensor_mul(zd_s[:, g], z_s[:, g], d_s[:, g])
                    nc.vector.tensor_add(hts[:, g], hs[:, g], zd_s[:, g])
                    eng = nc.gpsimd if gi < n_pg - 1 else nc.sync
                    eng.dma_start(hto[:, gg], hts[:, g])

            for it in range(n_total):
                rep, ci = divmod(it, n_chunks)
                lo = ci * chunk
                hi = lo + chunk
                if it == 0:
                    xs, hs = first_x, first_h
                else:
                    xs = io.tile([D, chunk], F16, tag="x")
                    nc.sync.dma_start(xs[:], xt[:, lo:hi])
                    hs = io.tile([D, chunk], F16, tag="h")
                    nc.sync.dma_start(hs[:], ht[:, lo:hi])

                if it == n_total - 1:
                    if pend is not None:
                        emit_blend(pend, granular=False)
                        pend = None
                    rh_tail = mid.tile([D, chunk], F16, tag="rh")
                    emit_tail_chunk(lo, hi, xs, hs)
                    break

                n_pg = chunk // psg
                # --- r gate (its PSUM buffer is recycled by ph) ---
                r_s = mid.tile([D, chunk], F16, tag="r")
                for gi in range(n_pg):
                    g = slice(gi * psg, (gi + 1) * psg)
                    pr = ps.tile([D, psg], F32, tag="p")
                    for si in range(psg // sub):
                        sl = slice(gi * psg + si * sub, gi * psg + (si + 1) * sub)
                        pl = slice(si * sub, (si + 1) * sub)
                        nc.tensor.matmul(pr[:, pl], w[2], xs[:, sl],
                                         start=True, stop=True)
                    nc.scalar.activation(r_s[:, g], pr[:], AF.Sigmoid,
                                         bias=bt[:, 1:2])

                # rh on DVE ASAP (ahead of any pending blend work)
                rh_s = mid.tile([D, chunk], F16, tag="rh")
                nc.vector.tensor_mul(rh_s[:], r_s[:], hs[:])

                # --- z gate ---
                z_s = mid.tile([D, chunk], F16, tag="z")
                for gi in range(n_pg):
                    g = slice(gi * psg, (gi + 1) * psg)
                    pz = ps.tile([D, psg], F32, tag="p")
                    for si in range(psg // sub):
                        sl = slice(gi * psg + si * sub, gi * psg + (si + 1) * sub)
                        pl = slice(si * sub, (si + 1) * sub)
                        nc.tensor.matmul(pz[:, pl], w[0], xs[:, sl],
                                         start=True, stop=False)
                        nc.tensor.matmul(pz[:, pl], w[1], hs[:, sl],
                                         start=False, stop=True)
                    nc.scalar.activation(z_s[:, g], pz[:], AF.Sigmoid,
                                         bias=bt[:, 0:1])

                # --- candidate ---
                hcs = io.tile([D, chunk], F16, tag="hco")
                for gi in range(n_pg):
                    g = slice(gi * psg, (gi + 1) * psg)
                    ph = ps.tile([D, psg], F32, tag="p")
                    for si in range(psg // sub):
                        sl = slice(gi * psg + si * sub, gi * psg + (si + 1) * sub)
                        pl = slice(si * sub, (si + 1) * sub)
                        nc.tensor.matmul(ph[:, pl], w[3], xs[:, sl],
                                         start=True, stop=False)
                        nc.tensor.matmul(ph[:, pl], w[4], rh_s[:, sl],
                                         start=False, stop=True)
                    nc.scalar.activation(hcs[:, g], ph[:], AF.Tanh,
                                         bias=bt[:, 2:3])
                nc.gpsimd.dma_start(hco[:, lo:hi], hcs[:])

                # blend of the PREVIOUS chunk (now that rh of this chunk is
                # queued on DVE)
                if pend is not None:
                    emit_blend(pend, granular=False)
                pend = (lo, hi, hs, z_s, hcs)

            if pend is not None:
                emit_blend(pend, granular=False)
    return nc


def make_nc(b_loc=B_LOC, chunk=2048, sub=512, nrep=1, **kw):
    nc = bacc.Bacc(
        "TRN2",
        target_bir_lowering=False,
        debug=False,
        enable_asserts=False,
        num_devices=N_CORES,
    )
    build_gru(nc, b_loc, chunk=chunk, sub=sub, nrep=nrep, **kw)
    nc.compile()
    return nc


def host_prep(x, h, W_update, U_update, B_update, W_reset, U_reset, B_reset, W_h, U_h, B_h):
    w_all = np.concatenate(
        [
            np.asarray(W_update, np.float32),
            np.asarray(U_update, np.float32),
            np.asarray(W_reset, np.float32) + np.asarray(U_reset, np.float32),
            np.asarray(W_h, np.float32).T,
            np.asarray(U_h, np.float32).T,
        ],
        axis=0,
    ).astype(np.float16)
    w_all = np.ascontiguousarray(w_all)
    b_all = np.stack(
        [
            np.asarray(B_update, np.float32).sum(axis=0),
            np.asarray(B_reset, np.float32).sum(axis=0),
            np.asarray(B_h, np.float32).sum(axis=0),
        ],
        axis=1,
    ).astype(np.float32)

    in_maps = []
    for c in range(N_CORES):
        rows = slice(c * B_LOC, (c + 1) * B_LOC)
        in_maps.append(
            {
                "xt": np.ascontiguousarray(
                    np.asarray(x, np.float32)[rows].T.astype(np.float16)),
                "ht": np.ascontiguousarray(
                    np.asarray(h, np.float32)[rows].T.astype(np.float16)),
                "w_all": w_all,
                "b_all": b_all,
            }
        )
    return in_maps


_NC_CACHE = {}


def kernel(**inputs):
    in_maps = host_prep(**inputs)
    if "nc" not in _NC_CACHE:
        _NC_CACHE["nc"] = make_nc()
    res = run_bass_kernel_spmd(_NC_CACHE["nc"], in_maps, list(range(N_CORES)))
    h_t = np.empty((B_FULL, D), np.float32)
    h_c = np.empty((B_FULL, D), np.float32)
    for c in range(N_CORES):
        rows = slice(c * B_LOC, (c + 1) * B_LOC)
        h_t[rows] = res.results[c]["ht_out"].T.astype(np.float32)
        h_c[rows] = res.results[c]["hc_out"].T.astype(np.float32)
    return h_t, h_c

